# revision 47
# baseline (speedup 1.0000x reference)
"""CKGAT knowledge-GAT kernel for 8 Trainium2 NeuronCores (Bass/Tile). v2

Math (per batch element b, per side in {user, item}, per layer i):
  pi   = leaky_relu(nh.(W@a1) + g2r[nr] + nt.(W@a3), 0.2)   [B,T,N]
  att  = softmax_N(pi)
  nei  = sum_n att * E[nh]                                   [B,T,D]
  emb  = elu((nei + E[t]) @ W).sum(T)                        [B,D]
  e_u  = mean_T E[user_h0] + sum_i emb_u_i
  e_v  = E[items] + mean_T E[item_h0] + sum_i emb_v_i
  out  = sigmoid(sum_d e_u * e_v)

Sharding: data-parallel over B (64 per core); entity table replicated as an
fp16 copy (host-converted) gathered in 512B 4-row blocks; the g3 = E.(W@a3)
table is precomputed sharded (12544 rows/core) and AllGathered.

Layout (per core): bt = b*32 + t in [0, 2048); partition p = bt//16,
l = bt%16, slot column c = l*8+n. Neighbor gathers land at [p, c] via the
int16 stream format (stream pos i -> [i%128, i//128]). Row extraction from
4-row blocks via copy + 3 copy_predicated; att-weighted sums and dot
reductions run in fp16 on DVE (2x mode) as pairwise-add cascades.
"""

import numpy as np

P = 128
BC, T, NN, D = 64, 32, 8, 64
BT = BC * T  # 2048
NE, NR = 100000, 32
NCORES = 8
EPC = 12544            # padded entity rows per core (8*12544 = 100352)
NE_PAD = NCORES * EPC  # 100352
G3_ROWS = NE_PAD // 64  # 1568 blocks of 64 scalars

_CACHE = {}


def _build():
    import concourse.bass as bass
    import concourse.bacc as bacc
    import concourse.mybir as mybir
    from concourse.tile import TileContext
    from concourse.masks import make_identity

    fp32 = mybir.dt.float32
    fp16 = mybir.dt.float16
    i32 = mybir.dt.int32
    i16 = mybir.dt.int16
    i8 = mybir.dt.int8
    Alu = mybir.AluOpType
    Act = mybir.ActivationFunctionType
    AxX = mybir.AxisListType.X

    def bc(ap_, *dims):
        """Append 0-stride broadcast dims to an AP."""
        return bass.AP(ap_.tensor, ap_.offset, list(ap_.ap) + [[0, d] for d in dims])

    def bcmid(t2d, n):
        """[128, M] tile -> [128, n(bcast), M] AP."""
        a = t2d[:]
        return bass.AP(a.tensor, a.offset, [list(a.ap[0]), [0, n], list(a.ap[1])])

    def dap(dram, dims, offset=0):
        """Arbitrary strided view of a DRAM tensor; dims = [(step, count), ...] els."""
        a = dram[:] if len(dram.shape) == 1 else dram[:, :]
        return bass.AP(a.tensor, a.offset + offset, [list(d) for d in dims])

    nc = bacc.Bacc("TRN2", target_bir_lowering=False, debug=False, dynamic_dma_scratch_size=16384)

    entH = nc.dram_tensor("entH", [NE, D], fp16, kind="ExternalInput")
    eslH = nc.dram_tensor("eslH", [EPC, D], fp16, kind="ExternalInput")
    rel = nc.dram_tensor("relation_emb", [NR, D], fp32, kind="ExternalInput")
    Wg = nc.dram_tensor("W_GAT", [D, D], fp32, kind="ExternalInput")
    ag = nc.dram_tensor("a_GAT", [3 * D, 1], fp32, kind="ExternalInput")
    items = nc.dram_tensor("items", [BC], i32, kind="ExternalInput")
    SIDES = ["u0", "u1", "i0", "i1"]
    nh_d = {s: nc.dram_tensor(f"nh_{s}", [BT, NN], i32, kind="ExternalInput") for s in SIDES}
    nr_d = {s: nc.dram_tensor(f"nr_{s}", [BT, NN], i32, kind="ExternalInput") for s in SIDES}
    nt_d = {s: nc.dram_tensor(f"nt_{s}", [BT, NN], i32, kind="ExternalInput") for s in SIDES}
    t_d = {s: nc.dram_tensor(f"t_{s}", [BT], i32, kind="ExternalInput") for s in SIDES}
    h0_d = {s: nc.dram_tensor(f"h0_{s}", [BT], i32, kind="ExternalInput") for s in ["u", "i"]}
    out_t = nc.dram_tensor("out", [1, BC], fp32, kind="ExternalOutput")
    DBG = {}
    import os
    if os.environ.get("KDBG"):
        DBG["C"] = nc.dram_tensor("dbg_C", [P, 1024 * NN], mybir.dt.float16, kind="ExternalOutput")
        DBG["s1"] = nc.dram_tensor("dbg_s1", [P, P], mybir.dt.float16, kind="ExternalOutput")
        DBG["g2"] = nc.dram_tensor("dbg_g2", [P, P], mybir.dt.float16, kind="ExternalOutput")
        DBG["g3"] = nc.dram_tensor("dbg_g3", [P, P], mybir.dt.float16, kind="ExternalOutput")
        DBG["att"] = nc.dram_tensor("dbg_att", [P, P], mybir.dt.float16, kind="ExternalOutput")
        DBG["X"] = nc.dram_tensor("dbg_X", [P, 1024], mybir.dt.float16, kind="ExternalOutput")
        DBG["accu"] = nc.dram_tensor("dbg_accu", [P, P], fp32, kind="ExternalOutput")
        DBG["trows"] = nc.dram_tensor("dbg_trows", [P, 1024], mybir.dt.float16, kind="ExternalOutput")
        DBG["xt0"] = nc.dram_tensor("dbg_xt0", [P, P], mybir.dt.float16, kind="ExternalOutput")
        DBG["e10"] = nc.dram_tensor("dbg_e10", [P, P], fp32, kind="ExternalOutput")

    g3in = nc.dram_tensor("g3in", [1, EPC], fp32)
    g3all = nc.dram_tensor("g3all", [G3_ROWS, 64], fp32)

    ent_blk = entH[:, :].rearrange("(a b) d -> a (b d)", b=4)  # [25000, 256] fp16

    with TileContext(nc) as tc:
        with (
            tc.tile_pool(name="const", bufs=1) as cp,
            tc.tile_pool(name="side", bufs=2) as sp,
            tc.tile_pool(name="scr", bufs=2) as scr,
            tc.tile_pool(name="q", bufs=2) as qp,
            tc.tile_pool(name="psum", bufs=2, space="PSUM") as pp,
            tc.tile_pool(name="psum1", bufs=1, space="PSUM") as pp1,
        ):
            # ---------------- constants / precompute ----------------
            id128 = cp.tile([P, P], fp32)
            make_identity(nc, id128[:])
            id16 = cp.tile([P, P], fp16)
            nc.vector.tensor_copy(out=id16[:], in_=id128[:])

            Wt_s = cp.tile([D, D], fp32)
            nc.sync.dma_start(out=Wt_s[:], in_=Wg[:, :])
            a1_s = cp.tile([D, 1], fp32)
            nc.sync.dma_start(out=a1_s[:], in_=ag[0:D, :])
            a2_s = cp.tile([D, 1], fp32)
            nc.sync.dma_start(out=a2_s[:], in_=ag[D:2 * D, :])
            a3_s = cp.tile([D, 1], fp32)
            nc.sync.dma_start(out=a3_s[:], in_=ag[2 * D:3 * D, :])
            rel_s = cp.tile([NR, D], fp32)
            nc.sync.dma_start(out=rel_s[:], in_=rel[:, :])

            # W transposed (PE)
            WT_p = pp1.tile([D, D], fp32, space="PSUM", tag="pp1t")
            nc.tensor.transpose(out=WT_p[:], in_=Wt_s[:], identity=id128[0:D, 0:D])
            WT_s = cp.tile([D, D], fp32)
            nc.vector.tensor_copy(out=WT_s[:], in_=WT_p[:])

            # w1 = W @ a1, w3 = W @ a3 as [1, 64] rows
            w1_p = pp1.tile([1, D], fp32, space="PSUM", tag="pp1t")
            nc.tensor.matmul(out=w1_p[:], lhsT=a1_s[:], rhs=WT_s[:], start=True, stop=True)
            w1_s = cp.tile([1, D], fp32)
            nc.vector.tensor_copy(out=w1_s[:], in_=w1_p[:])
            w3_p = pp1.tile([1, D], fp32, space="PSUM", tag="pp1t")
            nc.tensor.matmul(out=w3_p[:], lhsT=a3_s[:], rhs=WT_s[:], start=True, stop=True)
            w3_s = cp.tile([1, D], fp32)
            nc.vector.tensor_copy(out=w3_s[:], in_=w3_p[:])

            # g2r[r] = (R @ W) . a2  -> [1, 32]
            RT_p = pp1.tile([D, NR], fp32, space="PSUM", tag="pp1t")
            nc.tensor.transpose(out=RT_p[:], in_=rel_s[:], identity=id128[0:NR, 0:NR])
            RT_s = cp.tile([D, NR], fp32)
            nc.vector.tensor_copy(out=RT_s[:], in_=RT_p[:])
            RWT_p = pp1.tile([D, NR], fp32, space="PSUM", tag="pp1t")
            nc.tensor.matmul(out=RWT_p[:], lhsT=Wt_s[:], rhs=RT_s[:], start=True, stop=True)
            RWT_s = cp.tile([D, NR], fp32)
            nc.vector.tensor_copy(out=RWT_s[:], in_=RWT_p[:])
            g2r_p = pp1.tile([1, NR], fp32, space="PSUM", tag="pp1t")
            nc.tensor.matmul(out=g2r_p[:], lhsT=a2_s[:], rhs=RWT_s[:], start=True, stop=True)
            g2r_s = cp.tile([1, NR], fp32)
            nc.vector.tensor_copy(out=g2r_s[:], in_=g2r_p[:])

            # replicate w1/w3/g2r across all 128 partitions (PE broadcast)
            ones1 = cp.tile([1, P], fp32)
            nc.gpsimd.memset(ones1[:], 1.0)
            w1b_p = pp1.tile([P, D], fp32, space="PSUM", tag="pp1t")
            nc.tensor.matmul(out=w1b_p[:], lhsT=ones1[:], rhs=w1_s[:], start=True, stop=True)
            w1b16 = cp.tile([P, D], fp16)
            nc.vector.tensor_copy(out=w1b16[:], in_=w1b_p[:])
            w3b_p = pp1.tile([P, D], fp32, space="PSUM", tag="pp1t")
            nc.tensor.matmul(out=w3b_p[:], lhsT=ones1[:], rhs=w3_s[:], start=True, stop=True)
            w3b16 = cp.tile([P, D], fp16)
            nc.vector.tensor_copy(out=w3b16[:], in_=w3b_p[:])
            g2rb_p = pp1.tile([P, NR], fp32, space="PSUM", tag="pp1t")
            nc.tensor.matmul(out=g2rb_p[:], lhsT=ones1[:], rhs=g2r_s[:], start=True, stop=True)
            g2rb = cp.tile([P, NR], fp16)
            nc.vector.tensor_copy(out=g2rb[:], in_=g2rb_p[:])

            # block-diag(W, W) fp16 for the (nei+t) @ W matmul
            W2_s = cp.tile([P, P], fp32)
            nc.gpsimd.memset(W2_s[:], 0.0)
            nc.sync.dma_start(out=W2_s[0:D, 0:D], in_=Wg[:, :])
            nc.sync.dma_start(out=W2_s[D:P, D:P], in_=Wg[:, :])
            W2_16 = cp.tile([P, P], fp16)
            nc.vector.tensor_copy(out=W2_16[:], in_=W2_s[:])

            # stacked identity [[I],[I]] for summing partition halves via PE
            stack2 = cp.tile([P, D], fp32)
            nc.vector.tensor_copy(out=stack2[0:D, :], in_=id128[0:D, 0:D])
            nc.vector.tensor_copy(out=stack2[D:P, :], in_=id128[D:P, D:P])

            ones64 = cp.tile([D, 1], fp32)
            nc.gpsimd.memset(ones64[:], 1.0)

            # ---- g3 table: this core's 12544 rows of E . w3 (fp16 math,
            # fp32 table), then AllGather
            with tc.tile_pool(name="prep", bufs=1) as prp:
                NC_ = 7  # rows per partition per pass
                g3part = prp.tile([P, 14 * NC_], fp32, tag="g3part")
                for j2 in range(14):
                    echunk = prp.tile([P, NC_ * D], fp16, tag="echunk")
                    nc.sync.dma_start(
                        out=echunk[:],
                        in_=dap(eslH, [(14 * NC_ * D, P), (1, NC_ * D)], offset=j2 * NC_ * D))
                    g3tmp = prp.tile([P, NC_ * D], fp16, tag="g3tmp")
                    nc.vector.tensor_tensor(out=g3tmp[:].rearrange("p (j d) -> p j d", j=NC_),
                                            in0=echunk[:].rearrange("p (j d) -> p j d", j=NC_),
                                            in1=bcmid(w3b16, NC_), op=Alu.mult)
                    # pairwise-add cascade over d (fp16 until last round)
                    src = g3tmp[:].rearrange("p (j d) -> p j d", j=NC_)
                    w = D // 2
                    while w >= 1:
                        nxt_t = prp.tile([P, NC_ * w], fp16, tag=f"g3c_{w}")
                        nxt = nxt_t[:].rearrange("p (j d) -> p j d", j=NC_)
                        nc.vector.tensor_tensor(out=nxt, in0=src[:, :, 0:w],
                                                in1=src[:, :, w:2 * w], op=Alu.add)
                        src = nxt
                        w //= 2
                    nc.vector.tensor_copy(
                        out=g3part[:, j2 * NC_:(j2 + 1) * NC_], in_=src[:, :, 0])
                nc.scalar.dma_start(out=g3in[0, :].rearrange("(p j) -> p j", p=P), in_=g3part[:])
                nc.gpsimd.collective_compute(
                    "AllGather", Alu.bypass,
                    ins=[g3in[:, :]],
                    outs=[g3all[:, :].rearrange("a b -> (a b)").rearrange("(c e) -> c e", c=NCORES)],
                    replica_groups=[list(range(NCORES))],
                )

            # e_u / e_v accumulators [128 = (2 halves x 64 dout), 128 = (b, parity)]
            acc128 = {}
            for k in ["u", "v"]:
                acc128[k] = cp.tile([P, P], fp32, tag=f"acc_{k}", name=f"acc_{k}")
                nc.gpsimd.memset(acc128[k][:], 0.0)

            # ---------------- helpers ----------------
            def build_stream_n(dram, shift):
                """[BT, 8] int32 indices -> replicated int16 stream tile [128, 1024],
                stream pos i = (btlow*8+n)*128 + p, value = idx >> shift."""
                l3 = scr.tile([16, 1024], i32, tag="l3", bufs=1)
                nc.sync.dma_start(out=l3[:].rearrange("q (w l n) -> q w l n", w=8, l=16),
                                  in_=dap(dram, [(128, 16), (2048, 8), (8, 16), (1, 8)]))
                s32 = scr.tile([16, 1024], i32, tag="s32", bufs=1)
                nc.scalar.copy(s32[:].rearrange("q (x w) -> q x w", w=8),
                               l3[:].rearrange("q (w x) -> q x w", w=8))
                nc.vector.tensor_scalar(out=s32[:], in0=s32[:], scalar1=shift, scalar2=None,
                                        op0=Alu.logical_shift_right)
                s16 = scr.tile([16, 1024], i16, tag="s16", bufs=1)
                nc.vector.tensor_copy(out=s16[:], in_=s32[:])
                full = sp.tile([P, 1024], i16, tag=f"sf_{shift}")
                nc.sync.dma_start(out=full[0:16, :], in_=s16[:])
                for span in (16, 32, 64):
                    nc.sync.dma_start(out=full[span:2 * span, :], in_=full[0:span, :])
                return full

            def build_stream_t(dram, ncols=128, wq=8):
                """[BT] int32 -> replicated int16 stream [128, ncols], pos i = btlow*128+p,
                value = idx >> 2 (row-block ids)."""
                l3 = scr.tile([16, ncols], i32, tag="l3t")
                nc.sync.dma_start(out=l3[:].rearrange("q (w l) -> q w l", w=wq),
                                  in_=dap(dram, [(16, 16), (256, wq), (1, 16)]))
                s32 = scr.tile([16, ncols], i32, tag="s32t")
                nc.scalar.copy(s32[:].rearrange("q (x w) -> q x w", w=wq),
                               l3[:].rearrange("q (w x) -> q x w", w=wq))
                nc.vector.tensor_scalar(out=s32[:], in0=s32[:], scalar1=2, scalar2=None,
                                        op0=Alu.logical_shift_right)
                s16 = scr.tile([16, ncols], i16, tag="s16t")
                nc.vector.tensor_copy(out=s16[:], in_=s32[:])
                full = sp.tile([P, ncols], i16, tag="sft")
                nc.sync.dma_start(out=full[0:16, :], in_=s16[:])
                for span in (16, 32, 64):
                    nc.sync.dma_start(out=full[span:2 * span, :], in_=full[0:span, :])
                return full

            def rmask3(nat32, tag):
                """sub-row masks (idx&3)==k for k=1,2,3 from a natural-layout int32 tile."""
                rr = sp.tile(list(nat32.shape), i32, tag=f"rr_{tag}")
                nc.vector.tensor_scalar(out=rr[:], in0=nat32[:], scalar1=3, scalar2=None,
                                        op0=Alu.bitwise_and)
                ms = []
                for k in (1, 2, 3):
                    m = sp.tile(list(nat32.shape), i32, tag=f"rm{k}_{tag}")
                    nc.vector.tensor_scalar(out=m[:], in0=rr[:], scalar1=k, scalar2=None,
                                            op0=Alu.is_equal)
                    ms.append(m)
                return ms

            def extract_rows(gblk, masks, mslice, nslots, out_view, base_on_act=False):
                """gblk [128, nslots, 4(row), 64] fp16 -> out_view [128, nslots, 64]
                using per-slot sub-row masks. Predicated moves run on int32
                bitcast views (half the elements, same bytes)."""
                gv = gblk[:].rearrange("p (s r d) -> p s r d", s=nslots, r=4)
                if base_on_act:
                    nc.scalar.copy(out_view, gv[:, :, 0, :])
                else:
                    nc.vector.tensor_copy(out=out_view, in_=gv[:, :, 0, :])
                out32 = out_view.bitcast(i32)
                for k in (1, 2, 3):
                    nc.vector.copy_predicated(out=out32, mask=bc(mslice(masks[k - 1]), D // 2),
                                              data=gv[:, :, k, :].bitcast(i32))

            def bit_masks(nat32, bits, tag):
                ms = []
                for b in bits:
                    m = sp.tile(list(nat32.shape), i32, tag=f"bm{b}_{tag}")
                    nc.vector.tensor_scalar(out=m[:], in0=nat32[:], scalar1=b, scalar2=None,
                                            op0=Alu.bitwise_and)
                    ms.append(m)
                return ms

            # ---------------- layer-0 terms ----------------
            # mean_T E[user_h0] -> acc_u ; mean_T E[item_h0] -> acc_v
            for hs_, k in [("u", "u"), ("i", "v")]:
                s_h = build_stream_t(h0_d[hs_])
                nat_h = sp.tile([P, 16], i32, tag="nath")
                nc.sync.dma_start(out=nat_h[:], in_=h0_d[hs_][:].rearrange("(p l) -> p l", l=16))
                h_rm = rmask3(nat_h, "h0")
                gh = qp.tile([P, 16 * 4 * D], fp16, tag="gt", bufs=1)
                nc.gpsimd.dma_gather(
                    out_ap=gh[:].rearrange("p (kk e) -> p kk e", kk=16),
                    in_ap=ent_blk, idxs_ap=s_h[:], num_idxs=BT, num_idxs_reg=BT,
                    elem_size=256, single_packet=False)
                hrows = scr.tile([P, 16 * D], fp16, tag="hrows", bufs=1)
                extract_rows(gh, h_rm, lambda m: m[:], 16,
                             hrows[:].rearrange("p (s d) -> p s d", s=16))
                hrows32 = scr.tile([P, 16 * D], fp32, tag="hrows32", bufs=1)
                nc.scalar.mul(hrows32[:], hrows[:], 1.0 / T)
                ht_p = pp.tile([P, P], fp32, space="PSUM", tag="y")
                for ch in range(8):
                    nc.tensor.matmul(out=ht_p[:], lhsT=hrows32[:, 128 * ch:128 * ch + 128],
                                     rhs=id128[:], is_transpose=True,
                                     start=(ch == 0), stop=(ch == 7))
                nc.vector.tensor_tensor(out=acc128[k][:], in0=acc128[k][:], in1=ht_p[:],
                                        op=Alu.add)


            # ---------------- per-side processing ----------------
            for s in SIDES:
                acc = acc128["u" if s[0] == "u" else "v"]

                nat_nh = sp.tile([P, P], i32, tag="natnh")
                nc.sync.dma_start(out=nat_nh[:], in_=nh_d[s][:, :].rearrange("(p l) n -> p (l n)", l=16))
                nat_nt = sp.tile([P, P], i32, tag="natnt")
                nc.sync.dma_start(out=nat_nt[:], in_=nt_d[s][:, :].rearrange("(p l) n -> p (l n)", l=16))
                nat_nr = sp.tile([P, P], i32, tag="natnr")
                nc.sync.dma_start(out=nat_nr[:], in_=nr_d[s][:, :].rearrange("(p l) n -> p (l n)", l=16))
                nat_t = sp.tile([P, 16], i32, tag="natt")
                nc.sync.dma_start(out=nat_t[:], in_=t_d[s][:].rearrange("(p l) -> p l", l=16))

                s_nh = build_stream_n(nh_d[s], 2)        # row-block ids of nh
                s_nt6 = build_stream_n(nt_d[s], 6)       # g3 block stream: (nt >> 6)
                s_t = build_stream_t(t_d[s])

                nh_rm = rmask3(nat_nh, "nh")             # [128,128] each
                t_rm = rmask3(nat_t, "t")                # [128,16]
                nt_sub = sp.tile([P, P], i32, tag="ntsub")
                nc.vector.tensor_scalar(out=nt_sub[:], in0=nat_nt[:], scalar1=63, scalar2=None,
                                        op0=Alu.bitwise_and)
                nt_bits = bit_masks(nt_sub, [32, 16, 8, 4, 2, 1], "nt")
                nr_bits = bit_masks(nat_nr, [16, 8, 4, 2, 1], "nr")

                # g2r lookup for the whole side: [128, 128]
                rin = None
                width = 16
                lvl = 0
                while width >= 1:
                    rt = scr.tile([P, P * width], fp16, tag=f"tree_{P * width}", name=f"t2s_{width}")
                    rv = rt[:].rearrange("p (s e) -> p s e", s=P)
                    if lvl == 0:
                        a0 = g2rb[:]
                        lo = bass.AP(a0.tensor, a0.offset, [list(a0.ap[0]), [0, P], [1, 16]])
                        hi = bass.AP(a0.tensor, a0.offset + 16, [list(a0.ap[0]), [0, P], [1, 16]])
                    else:
                        lo = rin[:, :, 0:width]
                        hi = rin[:, :, width:2 * width]
                    nc.scalar.copy(rv, lo)
                    if width >= 2:
                        nc.vector.copy_predicated(out=rv.bitcast(i32),
                                                  mask=bc(nr_bits[lvl][:, :], width // 2),
                                                  data=hi.bitcast(i32))
                    else:
                        nc.vector.copy_predicated(out=rv, mask=bc(nr_bits[lvl][:, :], width),
                                                  data=hi)
                    rin = rv
                    width //= 2
                    lvl += 1
                g2side = rin  # [128, 128, 1]

                # C rows: all 8 q-chunk gathers + extraction first (feeds DVE asap)
                C = sp.tile([P, 8 * 16 * D], fp16, tag="C")
                for q in range(8):
                    qs = slice(16 * q, 16 * q + 16)
                    g = qp.tile([P, 16 * 4 * D], fp16, tag="G", bufs=2)
                    nc.gpsimd.dma_gather(
                        out_ap=g[:].rearrange("p (k e) -> p k e", k=16),
                        in_ap=ent_blk, idxs_ap=s_nh[:, 128 * q:128 * q + 128],
                        num_idxs=2048, num_idxs_reg=2048, elem_size=256, single_packet=False)
                    extract_rows(g, nh_rm, lambda m: m[:, qs], 16,
                                 C[:].rearrange("p (s d) -> p s d", s=128)[:, qs, :])

                # t rows for the whole side: [128, 16, 64] fp16
                gt = qp.tile([P, 16 * 4 * D], fp16, tag="gt", bufs=1)
                nc.gpsimd.dma_gather(
                    out_ap=gt[:].rearrange("p (k e) -> p k e", k=16),
                    in_ap=ent_blk, idxs_ap=s_t[:], num_idxs=BT, num_idxs_reg=BT,
                    elem_size=256, single_packet=False)
                trows = sp.tile([P, 16 * D], fp16, tag="trows")
                extract_rows(gt, t_rm, lambda m: m[:], 16,
                             trows[:].rearrange("p (s d) -> p s d", s=16))

                # g3 blocks + select tree per half-side
                g3side = sp.tile([P, P], fp16, tag="g3side")
                for half in range(2):
                    g3hb = scr.tile([P, 64 * 64], fp32, tag="g3hb", bufs=2)
                    nc.gpsimd.dma_gather(
                        out_ap=g3hb[:].rearrange("p (k e) -> p k e", k=64),
                        in_ap=g3all[:, :], idxs_ap=s_nt6[:, 512 * half:512 * half + 512],
                        num_idxs=8192, num_idxs_reg=8192, elem_size=64, single_packet=False)
                    # g3 select tree on the half-side: [128, 64, 64] -> [128, 64]
                    hs = slice(64 * half, 64 * half + 64)
                    tin = g3hb[:].rearrange("p (s e) -> p s e", s=64)
                    width = 32
                    lvl = 0
                    while width >= 1:
                        if width > 1:
                            tt = scr.tile([P, 64 * width], fp16, tag=f"tree_{64 * width}", name=f"t3_{width}")
                            tv3 = tt[:].rearrange("p (s e) -> p s e", s=64)
                        else:
                            tv3 = g3side[:, hs].rearrange("p (s e) -> p s e", s=64)
                        nc.scalar.copy(tv3, tin[:, :, 0:width])
                        if lvl >= 1 and width >= 2:
                            nc.vector.copy_predicated(
                                out=tv3.bitcast(i32), mask=bc(nt_bits[lvl][:, hs], width // 2),
                                data=tin[:, :, width:2 * width].bitcast(i32))
                        else:
                            nc.vector.copy_predicated(
                                out=tv3, mask=bc(nt_bits[lvl][:, hs], width),
                                data=tin[:, :, width:2 * width])
                        tin = tv3
                        width //= 2
                        lvl += 1

                # s1 = C . w1 per half-side (fp16 mult + pairwise cascade)
                s1f = sp.tile([P, P], fp16, tag="s1f")
                for half in range(2):
                    hs = slice(64 * half, 64 * half + 64)
                    tmp1 = scr.tile([P, 64 * D], fp16, tag="halfA", bufs=1)
                    nc.vector.tensor_tensor(
                        out=tmp1[:].rearrange("p (s d) -> p s d", s=64),
                        in0=C[:].rearrange("p (s d) -> p s d", s=128)[:, hs, :],
                        in1=bcmid(w1b16, 64), op=Alu.mult)
                    src = tmp1[:].rearrange("p (s d) -> p s d", s=64)
                    w = D // 2
                    while w >= 1:
                        if w > 1:
                            nxt_t = scr.tile([P, 64 * w], fp16, tag=f"s1c_{w}", bufs=1)
                            nxt = nxt_t[:].rearrange("p (s d) -> p s d", s=64)
                        else:
                            nxt = s1f[:, hs].rearrange("p (s d) -> p s d", s=64)
                        nc.vector.tensor_tensor(out=nxt, in0=src[:, :, 0:w],
                                                in1=src[:, :, w:2 * w], op=Alu.add)
                        src = nxt
                        w //= 2

                # pi = s1 + g2 + g3; leaky; softmax over n (f32 softmax path)
                pi = sp.tile([P, P], fp16, tag="pi")
                nc.vector.tensor_tensor(out=pi[:], in0=s1f[:], in1=g2side[:, :, 0], op=Alu.add)
                nc.vector.tensor_tensor(out=pi[:], in0=pi[:], in1=g3side[:], op=Alu.add)
                pi32 = scr.tile([P, P], fp32, tag="pi32", bufs=1)
                nc.vector.tensor_scalar(out=pi32[:], in0=pi[:], scalar1=0.2, scalar2=None,
                                        op0=Alu.mult)
                piL = scr.tile([P, P], fp32, tag="piL", bufs=1)
                nc.vector.tensor_copy(out=piL[:], in_=pi[:])
                nc.vector.tensor_tensor(out=piL[:], in0=piL[:], in1=pi32[:], op=Alu.max)
                ex = scr.tile([P, P], fp32, tag="ex32", bufs=1)
                nc.scalar.activation(ex[:], piL[:], Act.Exp)
                den = sp.tile([P, 16], fp32, tag="den")
                nc.vector.tensor_reduce(out=den[:], in_=ex[:].rearrange("p (l n) -> p l n", l=16),
                                        axis=AxX, op=Alu.add)
                rinv = sp.tile([P, 16], fp32, tag="rinv")
                nc.vector.reciprocal(out=rinv[:], in_=den[:])
                att32 = scr.tile([P, P], fp32, tag="att32", bufs=1)
                nc.vector.tensor_tensor(
                    out=att32[:].rearrange("p (l n) -> p l n", l=16),
                    in0=ex[:].rearrange("p (l n) -> p l n", l=16),
                    in1=bass.AP(rinv[:].tensor, rinv[:].offset,
                                [list(rinv[:].ap[0]), [1, 16], [0, 8]]),
                    op=Alu.mult)
                att16 = sp.tile([P, P], fp16, tag="att16")
                nc.vector.tensor_copy(out=att16[:], in_=att32[:])

                # weighted sum nei = sum_n att*C, X = nei + trows (per half)
                X = sp.tile([P, 16 * D], fp16, tag="X")
                for half in range(2):
                    hs = slice(64 * half, 64 * half + 64)
                    attx = scr.tile([P, 64 * D], fp16, tag="halfA", bufs=1)
                    nc.scalar.copy(
                        attx[:].rearrange("p (s d) -> p s d", s=64),
                        bc(att16[:, hs], D))
                    wtmp = scr.tile([P, 64 * D], fp16, tag="halfB", bufs=1)
                    nc.vector.tensor_tensor(
                        out=wtmp[:].rearrange("p (s d) -> p s d", s=64),
                        in0=C[:].rearrange("p (s d) -> p s d", s=128)[:, hs, :],
                        in1=attx[:].rearrange("p (s d) -> p s d", s=64), op=Alu.mult)
                    wv = wtmp[:].rearrange("p (l n d) -> p l n d", l=8, n=8)
                    r1 = scr.tile([P, 8 * 4 * D], fp16, tag="wr1", bufs=1)
                    r1v = r1[:].rearrange("p (l n d) -> p l n d", l=8, n=4)
                    nc.vector.tensor_tensor(out=r1v, in0=wv[:, :, 0:4, :],
                                            in1=wv[:, :, 4:8, :], op=Alu.add)
                    r2 = scr.tile([P, 8 * 2 * D], fp16, tag="wr2", bufs=1)
                    r2v = r2[:].rearrange("p (l n d) -> p l n d", l=8, n=2)
                    nc.vector.tensor_tensor(out=r2v, in0=r1v[:, :, 0:2, :],
                                            in1=r1v[:, :, 2:4, :], op=Alu.add)
                    xv = X[:].rearrange("p (l d) -> p l d", l=16)[:, 8 * half:8 * half + 8, :]
                    tv = trows[:].rearrange("p (l d) -> p l d", l=16)[:, 8 * half:8 * half + 8, :]
                    nc.vector.tensor_tensor(out=xv, in0=r2v[:, :, 0, :],
                                            in1=r2v[:, :, 1, :], op=Alu.add)
                    nc.vector.tensor_tensor(out=xv, in0=xv, in1=tv, op=Alu.add)

                if DBG and s == "u1":
                    nc.sync.dma_start(out=DBG["C"][:, :], in_=C[:])
                    nc.sync.dma_start(out=DBG["s1"][:, :], in_=s1f[:])
                    nc.sync.dma_start(out=DBG["g2"][:, :], in_=g2side[:, :, 0])
                    nc.sync.dma_start(out=DBG["g3"][:, :], in_=g3side[:])
                    nc.sync.dma_start(out=DBG["att"][:, :], in_=att16[:])
                    nc.sync.dma_start(out=DBG["X"][:, :], in_=X[:])
                    nc.sync.dma_start(out=DBG["trows"][:, :], in_=trows[:])

                # (X @ W) with elu, accumulate into acc128 [(2h x dout), (b, parity)]
                for q in range(8):
                    xt_p = pp.tile([P, P], fp16, space="PSUM", tag="xt")
                    nc.tensor.transpose(out=xt_p[:], in_=X[:, 128 * q:128 * q + 128],
                                        identity=id16[:])
                    xt16 = qp.tile([P, P], fp16, tag="xts")
                    nc.scalar.copy(xt16[:], xt_p[:])
                    y_p = pp.tile([P, P], fp32, space="PSUM", tag="y")
                    nc.tensor.matmul(out=y_p[:], lhsT=W2_16[:], rhs=xt16[:], start=True, stop=True)
                    e1 = qp.tile([P, P], fp32, tag="e1")
                    nc.scalar.activation(e1[:], y_p[:], Act.Exp)
                    r1a = qp.tile([P, P], fp32, tag="r1a")
                    nc.scalar.activation(r1a[:], y_p[:], Act.Relu)
                    # elu' = min(exp,1) + relu  (off by +1, corrected at the end)
                    nc.vector.tensor_scalar(out=e1[:], in0=e1[:], scalar1=1.0, scalar2=None,
                                            op0=Alu.min)
                    nc.vector.tensor_tensor(out=e1[:], in0=e1[:], in1=r1a[:], op=Alu.add)
                    if DBG and s == "u1" and q == 0:
                        nc.sync.dma_start(out=DBG["xt0"][:, :], in_=xt16[:])
                        nc.sync.dma_start(out=DBG["e10"][:, :], in_=e1[:])
                    nc.vector.tensor_tensor(out=acc[:], in0=acc[:], in1=e1[:], op=Alu.add)

            # E[items] -> added post-fold
            s_it = sp.tile([16, 4], i32, tag="sit32")
            nc.sync.dma_start(out=s_it[:], in_=items[:].rearrange("(w q) -> q w", w=4))
            nc.vector.tensor_scalar(out=s_it[:], in0=s_it[:], scalar1=2, scalar2=None,
                                    op0=Alu.logical_shift_right)
            s_it16 = sp.tile([16, 4], i16, tag="sit16")
            nc.vector.tensor_copy(out=s_it16[:], in_=s_it[:])
            s_itf = sp.tile([P, 4], i16, tag="sitf")
            for k in range(8):
                nc.sync.dma_start(out=s_itf[16 * k:16 * k + 16, :], in_=s_it16[:])
            nat_it = sp.tile([BC, 1], i32, tag="natit")
            nc.sync.dma_start(out=nat_it[:], in_=items[:, None])
            it_rm = rmask3(nat_it, "it")
            git = scr.tile([P, 4 * D], fp16, tag="git", bufs=1)
            nc.gpsimd.dma_gather(
                out_ap=git[:].rearrange("p (kk e) -> p kk e", kk=1),
                in_ap=ent_blk, idxs_ap=s_itf[:], num_idxs=BC, num_idxs_reg=BC,
                elem_size=256, single_packet=False)
            itrows = sp.tile([BC, D], fp16, tag="itrows")
            gitv = git[0:BC, :].rearrange("p (s r d) -> p s r d", s=1, r=4)
            nc.vector.tensor_copy(out=itrows[:].rearrange("p (s d) -> p s d", s=1),
                                  in_=gitv[:, :, 0, :])
            for kk in (1, 2, 3):
                nc.vector.copy_predicated(out=itrows[:].rearrange("p (s d) -> p s d", s=1),
                                          mask=bc(it_rm[kk - 1][:], D), data=gitv[:, :, kk, :])
            itrows32 = scr.tile([BC, D], fp32, tag="itrows32", bufs=1)
            nc.vector.tensor_copy(out=itrows32[:], in_=itrows[:])
            it_pt = pp.tile([P, P], fp32, space="PSUM", tag="y")
            it_p = it_pt[0:D, 0:BC]
            nc.tensor.transpose(out=it_p, in_=itrows32[:], identity=id128[0:BC, 0:BC])

            if DBG:
                nc.sync.dma_start(out=DBG["accu"][:, :], in_=acc128["u"][:])

            # ---------------- final: sigmoid(e_u . e_v) ----------------
            # fold acc128 halves (PE) then parity columns; subtract the 2T elu bias
            eu_p = pp1.tile([D, P], fp32, space="PSUM", tag="pp1f")
            nc.tensor.matmul(out=eu_p[:], lhsT=stack2[:], rhs=acc128["u"][:], start=True, stop=True)
            eu_sb = scr.tile([D, P], fp32, tag="fold_sb", bufs=1)
            nc.vector.tensor_copy(out=eu_sb[:], in_=eu_p[:])
            eu_s = cp.tile([D, BC], fp32, tag="eu_s")
            ev_half = eu_sb[:].rearrange("p (b two) -> p b two", two=2)
            nc.vector.tensor_tensor(out=eu_s[:], in0=ev_half[:, :, 0],
                                    in1=ev_half[:, :, 1], op=Alu.add)
            nc.vector.tensor_scalar(out=eu_s[:], in0=eu_s[:], scalar1=float(2 * T),
                                    scalar2=None, op0=Alu.subtract)
            ev_p = pp1.tile([D, P], fp32, space="PSUM", tag="pp1f")
            nc.tensor.matmul(out=ev_p[:], lhsT=stack2[:], rhs=acc128["v"][:], start=True, stop=True)
            ev_sb = scr.tile([D, P], fp32, tag="fold_sb", bufs=1)
            nc.vector.tensor_copy(out=ev_sb[:], in_=ev_p[:])
            ev_s = cp.tile([D, BC], fp32, tag="ev_s")
            ev_half2 = ev_sb[:].rearrange("p (b two) -> p b two", two=2)
            nc.vector.tensor_tensor(out=ev_s[:], in0=ev_half2[:, :, 0],
                                    in1=ev_half2[:, :, 1], op=Alu.add)
            nc.vector.tensor_scalar(out=ev_s[:], in0=ev_s[:], scalar1=float(2 * T),
                                    scalar2=None, op0=Alu.subtract)
            nc.vector.tensor_tensor(out=ev_s[:], in0=ev_s[:], in1=it_p, op=Alu.add)
            prod = scr.tile([D, BC], fp32, tag="prod", bufs=1)
            nc.vector.tensor_tensor(out=prod[:], in0=eu_s[:], in1=ev_s[:], op=Alu.mult)
            dot_p = pp1.tile([1, BC], fp32, space="PSUM", tag="pp1t")
            nc.tensor.matmul(out=dot_p[:], lhsT=ones64[:], rhs=prod[:], start=True, stop=True)
            sig = cp.tile([1, BC], fp32)
            nc.scalar.activation(sig[:], dot_p[:], Act.Sigmoid)
            nc.sync.dma_start(out=out_t[:, :], in_=sig[:])

    nc.compile()
    return nc


def _prep_inputs(inputs):
    """Build the 8 per-core input maps from full inputs."""
    f32 = np.float32
    ent = np.asarray(inputs["entity_emb"], f32)
    entH = np.ascontiguousarray(ent.astype(np.float16))
    rel = np.ascontiguousarray(np.asarray(inputs["relation_emb"], f32))
    Wg = np.ascontiguousarray(np.asarray(inputs["W_GAT"], f32))
    ag = np.ascontiguousarray(np.asarray(inputs["a_GAT"], f32))
    entH_pad = np.zeros((NE_PAD, D), np.float16)
    entH_pad[:NE] = entH

    def i32(x):
        return np.ascontiguousarray(np.asarray(x, np.int32))

    items = i32(inputs["items"])
    uh, ut = i32(inputs["user_h"]), i32(inputs["user_t"])
    unh, unr, unt = i32(inputs["user_nh"]), i32(inputs["user_nr"]), i32(inputs["user_nt"])
    ih, it_ = i32(inputs["item_h"]), i32(inputs["item_t"])
    inh, inr, int_ = i32(inputs["item_nh"]), i32(inputs["item_nr"]), i32(inputs["item_nt"])

    maps = []
    for c in range(NCORES):
        bs = slice(c * BC, (c + 1) * BC)
        m = {
            "entH": entH,
            "eslH": np.ascontiguousarray(entH_pad[c * EPC:(c + 1) * EPC]),
            "relation_emb": rel,
            "W_GAT": Wg,
            "a_GAT": ag,
            "items": items[bs],
            "h0_u": uh[0, bs].reshape(BT),
            "h0_i": ih[0, bs].reshape(BT),
        }
        for li in range(2):
            m[f"nh_u{li}"] = unh[li, bs].reshape(BT, NN)
            m[f"nr_u{li}"] = unr[li, bs].reshape(BT, NN)
            m[f"nt_u{li}"] = unt[li, bs].reshape(BT, NN)
            m[f"t_u{li}"] = ut[li, bs].reshape(BT)
            m[f"nh_i{li}"] = inh[li, bs].reshape(BT, NN)
            m[f"nr_i{li}"] = inr[li, bs].reshape(BT, NN)
            m[f"nt_i{li}"] = int_[li, bs].reshape(BT, NN)
            m[f"t_i{li}"] = it_[li, bs].reshape(BT)
        maps.append(m)
    return maps


def kernel(**inputs) -> np.ndarray:
    from concourse import bass_utils
    if "nc" not in _CACHE:
        _CACHE["nc"] = _build()
    nc = _CACHE["nc"]
    maps = _prep_inputs(inputs)
    res = bass_utils.run_bass_kernel_spmd(nc, maps, core_ids=list(range(NCORES)))
    return np.concatenate([res.results[c]["out"][0] for c in range(NCORES)]).astype(np.float32)


# revision 53
# speedup vs baseline: 1.0213x; 1.0213x over previous
"""CKGAT knowledge-GAT kernel for 8 Trainium2 NeuronCores (Bass/Tile). v2

Math (per batch element b, per side in {user, item}, per layer i):
  pi   = leaky_relu(nh.(W@a1) + g2r[nr] + nt.(W@a3), 0.2)   [B,T,N]
  att  = softmax_N(pi)
  nei  = sum_n att * E[nh]                                   [B,T,D]
  emb  = elu((nei + E[t]) @ W).sum(T)                        [B,D]
  e_u  = mean_T E[user_h0] + sum_i emb_u_i
  e_v  = E[items] + mean_T E[item_h0] + sum_i emb_v_i
  out  = sigmoid(sum_d e_u * e_v)

Sharding: data-parallel over B (64 per core); entity table replicated as an
fp16 copy (host-converted) gathered in 512B 4-row blocks; the g3 = E.(W@a3)
table is precomputed sharded (12544 rows/core) and AllGathered.

Layout (per core): bt = b*32 + t in [0, 2048); partition p = bt//16,
l = bt%16, slot column c = l*8+n. Neighbor gathers land at [p, c] via the
int16 stream format (stream pos i -> [i%128, i//128]). Row extraction from
4-row blocks via copy + 3 copy_predicated; att-weighted sums and dot
reductions run in fp16 on DVE (2x mode) as pairwise-add cascades.
"""

import numpy as np

P = 128
BC, T, NN, D = 64, 32, 8, 64
BT = BC * T  # 2048
NE, NR = 100000, 32
NCORES = 8
EPC = 12544            # padded entity rows per core (8*12544 = 100352)
NE_PAD = NCORES * EPC  # 100352
G3_ROWS = NE_PAD // 64  # 1568 blocks of 64 scalars

_CACHE = {}


def _build():
    import concourse.bass as bass
    import concourse.bacc as bacc
    import concourse.mybir as mybir
    from concourse.tile import TileContext
    from concourse.masks import make_identity

    fp32 = mybir.dt.float32
    fp16 = mybir.dt.float16
    i32 = mybir.dt.int32
    i16 = mybir.dt.int16
    i8 = mybir.dt.int8
    Alu = mybir.AluOpType
    Act = mybir.ActivationFunctionType
    AxX = mybir.AxisListType.X

    def bc(ap_, *dims):
        """Append 0-stride broadcast dims to an AP."""
        return bass.AP(ap_.tensor, ap_.offset, list(ap_.ap) + [[0, d] for d in dims])

    def bcmid(t2d, n):
        """[128, M] tile -> [128, n(bcast), M] AP."""
        a = t2d[:]
        return bass.AP(a.tensor, a.offset, [list(a.ap[0]), [0, n], list(a.ap[1])])

    def dap(dram, dims, offset=0):
        """Arbitrary strided view of a DRAM tensor; dims = [(step, count), ...] els."""
        a = dram[:] if len(dram.shape) == 1 else dram[:, :]
        return bass.AP(a.tensor, a.offset + offset, [list(d) for d in dims])

    nc = bacc.Bacc("TRN2", target_bir_lowering=False, debug=False, dynamic_dma_scratch_size=16384)

    entH = nc.dram_tensor("entH", [NE, D], fp16, kind="ExternalInput")
    eslH = nc.dram_tensor("eslH", [EPC, D], fp16, kind="ExternalInput")
    rel = nc.dram_tensor("relation_emb", [NR, D], fp32, kind="ExternalInput")
    Wg = nc.dram_tensor("W_GAT", [D, D], fp32, kind="ExternalInput")
    ag = nc.dram_tensor("a_GAT", [3 * D, 1], fp32, kind="ExternalInput")
    items = nc.dram_tensor("items", [BC], i32, kind="ExternalInput")
    SIDES = ["u0", "u1", "i0", "i1"]
    nh_d = {s: nc.dram_tensor(f"nh_{s}", [BT, NN], i32, kind="ExternalInput") for s in SIDES}
    nr_d = {s: nc.dram_tensor(f"nr_{s}", [BT, NN], i32, kind="ExternalInput") for s in SIDES}
    nt_d = {s: nc.dram_tensor(f"nt_{s}", [BT, NN], i32, kind="ExternalInput") for s in SIDES}
    t_d = {s: nc.dram_tensor(f"t_{s}", [BT], i32, kind="ExternalInput") for s in SIDES}
    h0_d = {s: nc.dram_tensor(f"h0_{s}", [BT], i32, kind="ExternalInput") for s in ["u", "i"]}
    out_t = nc.dram_tensor("out", [1, BC], fp32, kind="ExternalOutput")
    DBG = {}
    import os
    if os.environ.get("KDBG"):
        DBG["C"] = nc.dram_tensor("dbg_C", [P, 1024 * NN], mybir.dt.float16, kind="ExternalOutput")
        DBG["s1"] = nc.dram_tensor("dbg_s1", [P, P], mybir.dt.float16, kind="ExternalOutput")
        DBG["g2"] = nc.dram_tensor("dbg_g2", [P, P], mybir.dt.float16, kind="ExternalOutput")
        DBG["g3"] = nc.dram_tensor("dbg_g3", [P, P], mybir.dt.float16, kind="ExternalOutput")
        DBG["att"] = nc.dram_tensor("dbg_att", [P, P], mybir.dt.float16, kind="ExternalOutput")
        DBG["X"] = nc.dram_tensor("dbg_X", [P, 1024], mybir.dt.float16, kind="ExternalOutput")
        DBG["accu"] = nc.dram_tensor("dbg_accu", [P, P], fp32, kind="ExternalOutput")
        DBG["trows"] = nc.dram_tensor("dbg_trows", [P, 1024], mybir.dt.float16, kind="ExternalOutput")
        DBG["xt0"] = nc.dram_tensor("dbg_xt0", [P, P], mybir.dt.float16, kind="ExternalOutput")
        DBG["e10"] = nc.dram_tensor("dbg_e10", [P, P], fp32, kind="ExternalOutput")

    g3in = nc.dram_tensor("g3in", [1, EPC], fp32)
    g3all = nc.dram_tensor("g3all", [G3_ROWS, 64], fp32)

    ent_blk = entH[:, :].rearrange("(a b) d -> a (b d)", b=4)  # [25000, 256] fp16

    with TileContext(nc) as tc:
        with (
            tc.tile_pool(name="const", bufs=1) as cp,
            tc.tile_pool(name="side", bufs=2) as sp,
            tc.tile_pool(name="scr", bufs=2) as scr,
            tc.tile_pool(name="q", bufs=2) as qp,
            tc.tile_pool(name="psum", bufs=2, space="PSUM") as pp,
            tc.tile_pool(name="psum1", bufs=1, space="PSUM") as pp1,
        ):
            # ---------------- constants / precompute ----------------
            id128 = cp.tile([P, P], fp32)
            make_identity(nc, id128[:])
            id16 = cp.tile([P, P], fp16)
            nc.vector.tensor_copy(out=id16[:], in_=id128[:])

            Wt_s = cp.tile([D, D], fp32)
            nc.sync.dma_start(out=Wt_s[:], in_=Wg[:, :])
            a1_s = cp.tile([D, 1], fp32)
            nc.sync.dma_start(out=a1_s[:], in_=ag[0:D, :])
            a2_s = cp.tile([D, 1], fp32)
            nc.sync.dma_start(out=a2_s[:], in_=ag[D:2 * D, :])
            a3_s = cp.tile([D, 1], fp32)
            nc.sync.dma_start(out=a3_s[:], in_=ag[2 * D:3 * D, :])
            rel_s = cp.tile([NR, D], fp32)
            nc.sync.dma_start(out=rel_s[:], in_=rel[:, :])

            # W transposed (PE)
            WT_p = pp1.tile([D, D], fp32, space="PSUM", tag="pp1t")
            nc.tensor.transpose(out=WT_p[:], in_=Wt_s[:], identity=id128[0:D, 0:D])
            WT_s = cp.tile([D, D], fp32)
            nc.vector.tensor_copy(out=WT_s[:], in_=WT_p[:])

            # w1 = W @ a1, w3 = W @ a3 as [1, 64] rows
            w1_p = pp1.tile([1, D], fp32, space="PSUM", tag="pp1t")
            nc.tensor.matmul(out=w1_p[:], lhsT=a1_s[:], rhs=WT_s[:], start=True, stop=True)
            w1_s = cp.tile([1, D], fp32)
            nc.vector.tensor_copy(out=w1_s[:], in_=w1_p[:])
            w3_p = pp1.tile([1, D], fp32, space="PSUM", tag="pp1t")
            nc.tensor.matmul(out=w3_p[:], lhsT=a3_s[:], rhs=WT_s[:], start=True, stop=True)
            w3_s = cp.tile([1, D], fp32)
            nc.vector.tensor_copy(out=w3_s[:], in_=w3_p[:])

            # g2r[r] = (R @ W) . a2  -> [1, 32]
            RT_p = pp1.tile([D, NR], fp32, space="PSUM", tag="pp1t")
            nc.tensor.transpose(out=RT_p[:], in_=rel_s[:], identity=id128[0:NR, 0:NR])
            RT_s = cp.tile([D, NR], fp32)
            nc.vector.tensor_copy(out=RT_s[:], in_=RT_p[:])
            RWT_p = pp1.tile([D, NR], fp32, space="PSUM", tag="pp1t")
            nc.tensor.matmul(out=RWT_p[:], lhsT=Wt_s[:], rhs=RT_s[:], start=True, stop=True)
            RWT_s = cp.tile([D, NR], fp32)
            nc.vector.tensor_copy(out=RWT_s[:], in_=RWT_p[:])
            g2r_p = pp1.tile([1, NR], fp32, space="PSUM", tag="pp1t")
            nc.tensor.matmul(out=g2r_p[:], lhsT=a2_s[:], rhs=RWT_s[:], start=True, stop=True)
            g2r_s = cp.tile([1, NR], fp32)
            nc.vector.tensor_copy(out=g2r_s[:], in_=g2r_p[:])

            # replicate w1/w3/g2r across all 128 partitions (PE broadcast)
            ones1 = cp.tile([1, P], fp32)
            nc.gpsimd.memset(ones1[:], 1.0)
            w1b_p = pp1.tile([P, D], fp32, space="PSUM", tag="pp1t")
            nc.tensor.matmul(out=w1b_p[:], lhsT=ones1[:], rhs=w1_s[:], start=True, stop=True)
            w1b16 = cp.tile([P, D], fp16)
            nc.vector.tensor_copy(out=w1b16[:], in_=w1b_p[:])
            w3b_p = pp1.tile([P, D], fp32, space="PSUM", tag="pp1t")
            nc.tensor.matmul(out=w3b_p[:], lhsT=ones1[:], rhs=w3_s[:], start=True, stop=True)
            w3b16 = cp.tile([P, D], fp16)
            nc.vector.tensor_copy(out=w3b16[:], in_=w3b_p[:])
            g2rb_p = pp1.tile([P, NR], fp32, space="PSUM", tag="pp1t")
            nc.tensor.matmul(out=g2rb_p[:], lhsT=ones1[:], rhs=g2r_s[:], start=True, stop=True)
            g2rb = cp.tile([P, NR], fp16)
            nc.vector.tensor_copy(out=g2rb[:], in_=g2rb_p[:])

            # block-diag(W, W) fp16 for the (nei+t) @ W matmul
            W2_s = cp.tile([P, P], fp32)
            nc.gpsimd.memset(W2_s[:], 0.0)
            nc.sync.dma_start(out=W2_s[0:D, 0:D], in_=Wg[:, :])
            nc.sync.dma_start(out=W2_s[D:P, D:P], in_=Wg[:, :])
            W2_16 = cp.tile([P, P], fp16)
            nc.vector.tensor_copy(out=W2_16[:], in_=W2_s[:])

            # stacked identity [[I],[I]] for summing partition halves via PE
            stack2 = cp.tile([P, D], fp32)
            nc.vector.tensor_copy(out=stack2[0:D, :], in_=id128[0:D, 0:D])
            nc.vector.tensor_copy(out=stack2[D:P, :], in_=id128[D:P, D:P])

            ones64 = cp.tile([D, 1], fp32)
            nc.gpsimd.memset(ones64[:], 1.0)

            # ---- g3 table: this core's 12544 rows of E . w3 (fp16 math,
            # fp32 table), then AllGather
            with tc.tile_pool(name="prep", bufs=1) as prp:
                NC_ = 7  # rows per partition per pass
                g3part = prp.tile([P, 14 * NC_], fp32, tag="g3part")
                for j2 in range(14):
                    echunk = prp.tile([P, NC_ * D], fp16, tag="echunk")
                    nc.sync.dma_start(
                        out=echunk[:],
                        in_=dap(eslH, [(14 * NC_ * D, P), (1, NC_ * D)], offset=j2 * NC_ * D))
                    g3tmp = prp.tile([P, NC_ * D], fp16, tag="g3tmp")
                    nc.vector.tensor_tensor(out=g3tmp[:].rearrange("p (j d) -> p j d", j=NC_),
                                            in0=echunk[:].rearrange("p (j d) -> p j d", j=NC_),
                                            in1=bcmid(w3b16, NC_), op=Alu.mult)
                    # pairwise-add cascade over d (fp16 until last round)
                    src = g3tmp[:].rearrange("p (j d) -> p j d", j=NC_)
                    w = D // 2
                    while w >= 1:
                        nxt_t = prp.tile([P, NC_ * w], fp16, tag=f"g3c_{w}")
                        nxt = nxt_t[:].rearrange("p (j d) -> p j d", j=NC_)
                        nc.vector.tensor_tensor(out=nxt, in0=src[:, :, 0:w],
                                                in1=src[:, :, w:2 * w], op=Alu.add)
                        src = nxt
                        w //= 2
                    nc.vector.tensor_copy(
                        out=g3part[:, j2 * NC_:(j2 + 1) * NC_], in_=src[:, :, 0])
                nc.scalar.dma_start(out=g3in[0, :].rearrange("(p j) -> p j", p=P), in_=g3part[:])
                nc.gpsimd.collective_compute(
                    "AllGather", Alu.bypass,
                    ins=[g3in[:, :]],
                    outs=[g3all[:, :].rearrange("a b -> (a b)").rearrange("(c e) -> c e", c=NCORES)],
                    replica_groups=[list(range(NCORES))],
                )

            # e_u / e_v accumulators [128 = (2 halves x 64 dout), 128 = (b, parity)]
            acc128 = {}
            for k in ["u", "v"]:
                acc128[k] = cp.tile([P, P], fp32, tag=f"acc_{k}", name=f"acc_{k}")
                nc.gpsimd.memset(acc128[k][:], 0.0)

            # ---------------- helpers ----------------
            def build_stream_n(dram, shift):
                """[BT, 8] int32 indices -> replicated int16 stream tile [128, 1024],
                stream pos i = (btlow*8+n)*128 + p, value = idx >> shift."""
                l3 = scr.tile([16, 1024], i32, tag="l3", bufs=1)
                nc.sync.dma_start(out=l3[:].rearrange("q (w l n) -> q w l n", w=8, l=16),
                                  in_=dap(dram, [(128, 16), (2048, 8), (8, 16), (1, 8)]))
                s32 = scr.tile([16, 1024], i32, tag="s32", bufs=1)
                nc.scalar.copy(s32[:].rearrange("q (x w) -> q x w", w=8),
                               l3[:].rearrange("q (w x) -> q x w", w=8))
                nc.vector.tensor_scalar(out=s32[:], in0=s32[:], scalar1=shift, scalar2=None,
                                        op0=Alu.logical_shift_right)
                s16 = scr.tile([16, 1024], i16, tag="s16", bufs=1)
                nc.vector.tensor_copy(out=s16[:], in_=s32[:])
                full = sp.tile([P, 1024], i16, tag=f"sf_{shift}")
                nc.sync.dma_start(out=full[0:16, :], in_=s16[:])
                for span in (16, 32, 64):
                    nc.sync.dma_start(out=full[span:2 * span, :], in_=full[0:span, :])
                return full

            def build_stream_t(dram, ncols=128, wq=8):
                """[BT] int32 -> replicated int16 stream [128, ncols], pos i = btlow*128+p,
                value = idx >> 2 (row-block ids)."""
                l3 = scr.tile([16, ncols], i32, tag="l3t")
                nc.sync.dma_start(out=l3[:].rearrange("q (w l) -> q w l", w=wq),
                                  in_=dap(dram, [(16, 16), (256, wq), (1, 16)]))
                s32 = scr.tile([16, ncols], i32, tag="s32t")
                nc.scalar.copy(s32[:].rearrange("q (x w) -> q x w", w=wq),
                               l3[:].rearrange("q (w x) -> q x w", w=wq))
                nc.vector.tensor_scalar(out=s32[:], in0=s32[:], scalar1=2, scalar2=None,
                                        op0=Alu.logical_shift_right)
                s16 = scr.tile([16, ncols], i16, tag="s16t")
                nc.vector.tensor_copy(out=s16[:], in_=s32[:])
                full = sp.tile([P, ncols], i16, tag="sft")
                nc.sync.dma_start(out=full[0:16, :], in_=s16[:])
                for span in (16, 32, 64):
                    nc.sync.dma_start(out=full[span:2 * span, :], in_=full[0:span, :])
                return full

            def rmask3(nat32, tag):
                """sub-row masks (idx&3)==k for k=1,2,3 from a natural-layout int32 tile."""
                rr = sp.tile(list(nat32.shape), i32, tag=f"rr_{tag}")
                nc.vector.tensor_scalar(out=rr[:], in0=nat32[:], scalar1=3, scalar2=None,
                                        op0=Alu.bitwise_and)
                ms = []
                for k in (1, 2, 3):
                    m = sp.tile(list(nat32.shape), i32, tag=f"rm{k}_{tag}")
                    nc.vector.tensor_scalar(out=m[:], in0=rr[:], scalar1=k, scalar2=None,
                                            op0=Alu.is_equal)
                    ms.append(m)
                return ms

            def extract_rows(gblk, masks, mslice, nslots, out_view, base_on_act=False):
                """gblk [128, nslots, 4(row), 64] fp16 -> out_view [128, nslots, 64]
                using per-slot sub-row masks. Predicated moves run on int32
                bitcast views (half the elements, same bytes)."""
                gv = gblk[:].rearrange("p (s r d) -> p s r d", s=nslots, r=4)
                if base_on_act:
                    nc.scalar.copy(out_view, gv[:, :, 0, :])
                else:
                    nc.vector.tensor_copy(out=out_view, in_=gv[:, :, 0, :])
                out32 = out_view.bitcast(i32)
                for k in (1, 2, 3):
                    nc.vector.copy_predicated(out=out32, mask=bc(mslice(masks[k - 1]), D // 2),
                                              data=gv[:, :, k, :].bitcast(i32))

            def bit_masks(nat32, bits, tag):
                ms = []
                for b in bits:
                    m = sp.tile(list(nat32.shape), i32, tag=f"bm{b}_{tag}")
                    nc.vector.tensor_scalar(out=m[:], in0=nat32[:], scalar1=b, scalar2=None,
                                            op0=Alu.bitwise_and)
                    ms.append(m)
                return ms

            # ---------------- layer-0 terms ----------------
            # mean_T E[user_h0] -> acc_u ; mean_T E[item_h0] -> acc_v
            for hs_, k in [("u", "u"), ("i", "v")]:
                s_h = build_stream_t(h0_d[hs_])
                nat_h = sp.tile([P, 16], i32, tag="nath")
                nc.sync.dma_start(out=nat_h[:], in_=h0_d[hs_][:].rearrange("(p l) -> p l", l=16))
                h_rm = rmask3(nat_h, "h0")
                gh = qp.tile([P, 16 * 4 * D], fp16, tag="gt", bufs=1)
                nc.gpsimd.dma_gather(
                    out_ap=gh[:].rearrange("p (kk e) -> p kk e", kk=16),
                    in_ap=ent_blk, idxs_ap=s_h[:], num_idxs=BT, num_idxs_reg=BT,
                    elem_size=256, single_packet=False)
                hrows = scr.tile([P, 16 * D], fp16, tag="hrows", bufs=1)
                extract_rows(gh, h_rm, lambda m: m[:], 16,
                             hrows[:].rearrange("p (s d) -> p s d", s=16),
                             base_on_act=True)
                hrows32 = scr.tile([P, 16 * D], fp32, tag="hrows32", bufs=1)
                nc.scalar.mul(hrows32[:], hrows[:], 1.0 / T)
                ht_p = pp.tile([P, P], fp32, space="PSUM", tag="y")
                for ch in range(8):
                    nc.tensor.matmul(out=ht_p[:], lhsT=hrows32[:, 128 * ch:128 * ch + 128],
                                     rhs=id128[:], is_transpose=True,
                                     start=(ch == 0), stop=(ch == 7))
                nc.vector.tensor_tensor(out=acc128[k][:], in0=acc128[k][:], in1=ht_p[:],
                                        op=Alu.add)


            # ---------------- per-side processing ----------------
            for s in SIDES:
                acc = acc128["u" if s[0] == "u" else "v"]

                nat_nh = sp.tile([P, P], i32, tag="natnh")
                nc.sync.dma_start(out=nat_nh[:], in_=nh_d[s][:, :].rearrange("(p l) n -> p (l n)", l=16))
                nat_nt = sp.tile([P, P], i32, tag="natnt")
                nc.sync.dma_start(out=nat_nt[:], in_=nt_d[s][:, :].rearrange("(p l) n -> p (l n)", l=16))
                nat_nr = sp.tile([P, P], i32, tag="natnr")
                nc.sync.dma_start(out=nat_nr[:], in_=nr_d[s][:, :].rearrange("(p l) n -> p (l n)", l=16))
                nat_t = sp.tile([P, 16], i32, tag="natt")
                nc.sync.dma_start(out=nat_t[:], in_=t_d[s][:].rearrange("(p l) -> p l", l=16))

                s_nh = build_stream_n(nh_d[s], 2)        # row-block ids of nh
                s_nt6 = build_stream_n(nt_d[s], 6)       # g3 block stream: (nt >> 6)
                s_t = build_stream_t(t_d[s])

                nh_rm = rmask3(nat_nh, "nh")             # [128,128] each
                t_rm = rmask3(nat_t, "t")                # [128,16]
                nt_sub = sp.tile([P, P], i32, tag="ntsub")
                nc.vector.tensor_scalar(out=nt_sub[:], in0=nat_nt[:], scalar1=63, scalar2=None,
                                        op0=Alu.bitwise_and)
                nt_bits = bit_masks(nt_sub, [32, 16, 8, 4, 2, 1], "nt")
                nr_bits = bit_masks(nat_nr, [16, 8, 4, 2, 1], "nr")

                # g2r lookup for the whole side: [128, 128]
                rin = None
                width = 16
                lvl = 0
                while width >= 1:
                    rt = scr.tile([P, P * width], fp16, tag=f"tree_{P * width}", name=f"t2s_{width}")
                    rv = rt[:].rearrange("p (s e) -> p s e", s=P)
                    if lvl == 0:
                        a0 = g2rb[:]
                        lo = bass.AP(a0.tensor, a0.offset, [list(a0.ap[0]), [0, P], [1, 16]])
                        hi = bass.AP(a0.tensor, a0.offset + 16, [list(a0.ap[0]), [0, P], [1, 16]])
                    else:
                        lo = rin[:, :, 0:width]
                        hi = rin[:, :, width:2 * width]
                    nc.scalar.copy(rv, lo)
                    if width >= 2:
                        nc.vector.copy_predicated(out=rv.bitcast(i32),
                                                  mask=bc(nr_bits[lvl][:, :], width // 2),
                                                  data=hi.bitcast(i32))
                    else:
                        nc.vector.copy_predicated(out=rv, mask=bc(nr_bits[lvl][:, :], width),
                                                  data=hi)
                    rin = rv
                    width //= 2
                    lvl += 1
                g2side = rin  # [128, 128, 1]

                # C rows: all 8 q-chunk gathers + extraction first (feeds DVE asap)
                C = sp.tile([P, 8 * 16 * D], fp16, tag="C")
                for q in range(8):
                    qs = slice(16 * q, 16 * q + 16)
                    g = qp.tile([P, 16 * 4 * D], fp16, tag="G", bufs=2)
                    nc.gpsimd.dma_gather(
                        out_ap=g[:].rearrange("p (k e) -> p k e", k=16),
                        in_ap=ent_blk, idxs_ap=s_nh[:, 128 * q:128 * q + 128],
                        num_idxs=2048, num_idxs_reg=2048, elem_size=256, single_packet=False)
                    extract_rows(g, nh_rm, lambda m: m[:, qs], 16,
                                 C[:].rearrange("p (s d) -> p s d", s=128)[:, qs, :],
                                 base_on_act=True)

                # t rows for the whole side: [128, 16, 64] fp16
                gt = qp.tile([P, 16 * 4 * D], fp16, tag="gt", bufs=1)
                nc.gpsimd.dma_gather(
                    out_ap=gt[:].rearrange("p (k e) -> p k e", k=16),
                    in_ap=ent_blk, idxs_ap=s_t[:], num_idxs=BT, num_idxs_reg=BT,
                    elem_size=256, single_packet=False)
                trows = sp.tile([P, 16 * D], fp16, tag="trows")
                extract_rows(gt, t_rm, lambda m: m[:], 16,
                             trows[:].rearrange("p (s d) -> p s d", s=16),
                             base_on_act=True)

                # g3 blocks + select tree per half-side
                g3side = sp.tile([P, P], fp16, tag="g3side")
                for half in range(2):
                    g3hb = scr.tile([P, 64 * 64], fp32, tag="g3hb", bufs=2)
                    nc.gpsimd.dma_gather(
                        out_ap=g3hb[:].rearrange("p (k e) -> p k e", k=64),
                        in_ap=g3all[:, :], idxs_ap=s_nt6[:, 512 * half:512 * half + 512],
                        num_idxs=8192, num_idxs_reg=8192, elem_size=64, single_packet=False)
                    # g3 select tree on the half-side: [128, 64, 64] -> [128, 64]
                    hs = slice(64 * half, 64 * half + 64)
                    tin = g3hb[:].rearrange("p (s e) -> p s e", s=64)
                    width = 32
                    lvl = 0
                    while width >= 1:
                        if width > 1:
                            tt = scr.tile([P, 64 * width], fp16, tag=f"tree_{64 * width}", name=f"t3_{width}")
                            tv3 = tt[:].rearrange("p (s e) -> p s e", s=64)
                        else:
                            tv3 = g3side[:, hs].rearrange("p (s e) -> p s e", s=64)
                        nc.scalar.copy(tv3, tin[:, :, 0:width])
                        if lvl >= 1 and width >= 2:
                            nc.vector.copy_predicated(
                                out=tv3.bitcast(i32), mask=bc(nt_bits[lvl][:, hs], width // 2),
                                data=tin[:, :, width:2 * width].bitcast(i32))
                        else:
                            nc.vector.copy_predicated(
                                out=tv3, mask=bc(nt_bits[lvl][:, hs], width),
                                data=tin[:, :, width:2 * width])
                        tin = tv3
                        width //= 2
                        lvl += 1

                # s1 = C . w1 per half-side (fp16 mult + pairwise cascade)
                s1f = sp.tile([P, P], fp16, tag="s1f")
                for half in range(2):
                    hs = slice(64 * half, 64 * half + 64)
                    tmp1 = scr.tile([P, 64 * D], fp16, tag="halfA", bufs=1)
                    nc.vector.tensor_tensor(
                        out=tmp1[:].rearrange("p (s d) -> p s d", s=64),
                        in0=C[:].rearrange("p (s d) -> p s d", s=128)[:, hs, :],
                        in1=bcmid(w1b16, 64), op=Alu.mult)
                    src = tmp1[:].rearrange("p (s d) -> p s d", s=64)
                    w = D // 2
                    while w >= 1:
                        if w > 1:
                            nxt_t = scr.tile([P, 64 * w], fp16, tag=f"s1c_{w}", bufs=1)
                            nxt = nxt_t[:].rearrange("p (s d) -> p s d", s=64)
                        else:
                            nxt = s1f[:, hs].rearrange("p (s d) -> p s d", s=64)
                        nc.vector.tensor_tensor(out=nxt, in0=src[:, :, 0:w],
                                                in1=src[:, :, w:2 * w], op=Alu.add)
                        src = nxt
                        w //= 2

                # pi = s1 + g2 + g3; leaky; softmax over n (f32 softmax path)
                pi = sp.tile([P, P], fp16, tag="pi")
                nc.vector.tensor_tensor(out=pi[:], in0=s1f[:], in1=g2side[:, :, 0], op=Alu.add)
                nc.vector.tensor_tensor(out=pi[:], in0=pi[:], in1=g3side[:], op=Alu.add)
                pi32 = scr.tile([P, P], fp32, tag="pi32", bufs=1)
                nc.vector.tensor_scalar(out=pi32[:], in0=pi[:], scalar1=0.2, scalar2=None,
                                        op0=Alu.mult)
                piL = scr.tile([P, P], fp32, tag="piL", bufs=1)
                nc.vector.tensor_copy(out=piL[:], in_=pi[:])
                nc.vector.tensor_tensor(out=piL[:], in0=piL[:], in1=pi32[:], op=Alu.max)
                ex = scr.tile([P, P], fp32, tag="ex32", bufs=1)
                nc.scalar.activation(ex[:], piL[:], Act.Exp)
                den = sp.tile([P, 16], fp32, tag="den")
                nc.vector.tensor_reduce(out=den[:], in_=ex[:].rearrange("p (l n) -> p l n", l=16),
                                        axis=AxX, op=Alu.add)
                rinv = sp.tile([P, 16], fp32, tag="rinv")
                nc.vector.reciprocal(out=rinv[:], in_=den[:])
                att32 = scr.tile([P, P], fp32, tag="att32", bufs=1)
                nc.vector.tensor_tensor(
                    out=att32[:].rearrange("p (l n) -> p l n", l=16),
                    in0=ex[:].rearrange("p (l n) -> p l n", l=16),
                    in1=bass.AP(rinv[:].tensor, rinv[:].offset,
                                [list(rinv[:].ap[0]), [1, 16], [0, 8]]),
                    op=Alu.mult)
                att16 = sp.tile([P, P], fp16, tag="att16")
                nc.vector.tensor_copy(out=att16[:], in_=att32[:])

                # weighted sum nei = sum_n att*C, X = nei + trows (per half)
                X = sp.tile([P, 16 * D], fp16, tag="X")
                for half in range(2):
                    hs = slice(64 * half, 64 * half + 64)
                    attx = scr.tile([P, 64 * D], fp16, tag="halfA", bufs=1)
                    nc.scalar.copy(
                        attx[:].rearrange("p (s d) -> p s d", s=64),
                        bc(att16[:, hs], D))
                    wtmp = scr.tile([P, 64 * D], fp16, tag="halfB", bufs=1)
                    nc.vector.tensor_tensor(
                        out=wtmp[:].rearrange("p (s d) -> p s d", s=64),
                        in0=C[:].rearrange("p (s d) -> p s d", s=128)[:, hs, :],
                        in1=attx[:].rearrange("p (s d) -> p s d", s=64), op=Alu.mult)
                    wv = wtmp[:].rearrange("p (l n d) -> p l n d", l=8, n=8)
                    r1 = scr.tile([P, 8 * 4 * D], fp16, tag="wr1", bufs=1)
                    r1v = r1[:].rearrange("p (l n d) -> p l n d", l=8, n=4)
                    nc.vector.tensor_tensor(out=r1v, in0=wv[:, :, 0:4, :],
                                            in1=wv[:, :, 4:8, :], op=Alu.add)
                    r2 = scr.tile([P, 8 * 2 * D], fp16, tag="wr2", bufs=1)
                    r2v = r2[:].rearrange("p (l n d) -> p l n d", l=8, n=2)
                    nc.vector.tensor_tensor(out=r2v, in0=r1v[:, :, 0:2, :],
                                            in1=r1v[:, :, 2:4, :], op=Alu.add)
                    xv = X[:].rearrange("p (l d) -> p l d", l=16)[:, 8 * half:8 * half + 8, :]
                    tv = trows[:].rearrange("p (l d) -> p l d", l=16)[:, 8 * half:8 * half + 8, :]
                    nc.vector.tensor_tensor(out=xv, in0=r2v[:, :, 0, :],
                                            in1=r2v[:, :, 1, :], op=Alu.add)
                    nc.vector.tensor_tensor(out=xv, in0=xv, in1=tv, op=Alu.add)

                if DBG and s == "u1":
                    nc.sync.dma_start(out=DBG["C"][:, :], in_=C[:])
                    nc.sync.dma_start(out=DBG["s1"][:, :], in_=s1f[:])
                    nc.sync.dma_start(out=DBG["g2"][:, :], in_=g2side[:, :, 0])
                    nc.sync.dma_start(out=DBG["g3"][:, :], in_=g3side[:])
                    nc.sync.dma_start(out=DBG["att"][:, :], in_=att16[:])
                    nc.sync.dma_start(out=DBG["X"][:, :], in_=X[:])
                    nc.sync.dma_start(out=DBG["trows"][:, :], in_=trows[:])

                # (X @ W) with elu, accumulate into acc128 [(2h x dout), (b, parity)]
                for q in range(8):
                    xt_p = pp.tile([P, P], fp16, space="PSUM", tag="xt")
                    nc.tensor.transpose(out=xt_p[:], in_=X[:, 128 * q:128 * q + 128],
                                        identity=id16[:])
                    xt16 = qp.tile([P, P], fp16, tag="xts")
                    nc.scalar.copy(xt16[:], xt_p[:])
                    y_p = pp.tile([P, P], fp32, space="PSUM", tag="y")
                    nc.tensor.matmul(out=y_p[:], lhsT=W2_16[:], rhs=xt16[:], start=True, stop=True)
                    e1 = qp.tile([P, P], fp32, tag="e1")
                    nc.scalar.activation(e1[:], y_p[:], Act.Exp)
                    r1a = qp.tile([P, P], fp32, tag="r1a")
                    nc.scalar.activation(r1a[:], y_p[:], Act.Relu)
                    # elu' = min(exp,1) + relu  (off by +1, corrected at the end)
                    nc.vector.tensor_scalar(out=e1[:], in0=e1[:], scalar1=1.0, scalar2=None,
                                            op0=Alu.min)
                    nc.vector.tensor_tensor(out=e1[:], in0=e1[:], in1=r1a[:], op=Alu.add)
                    if DBG and s == "u1" and q == 0:
                        nc.sync.dma_start(out=DBG["xt0"][:, :], in_=xt16[:])
                        nc.sync.dma_start(out=DBG["e10"][:, :], in_=e1[:])
                    nc.vector.tensor_tensor(out=acc[:], in0=acc[:], in1=e1[:], op=Alu.add)

            # E[items] -> added post-fold
            s_it = sp.tile([16, 4], i32, tag="sit32")
            nc.sync.dma_start(out=s_it[:], in_=items[:].rearrange("(w q) -> q w", w=4))
            nc.vector.tensor_scalar(out=s_it[:], in0=s_it[:], scalar1=2, scalar2=None,
                                    op0=Alu.logical_shift_right)
            s_it16 = sp.tile([16, 4], i16, tag="sit16")
            nc.vector.tensor_copy(out=s_it16[:], in_=s_it[:])
            s_itf = sp.tile([P, 4], i16, tag="sitf")
            for k in range(8):
                nc.sync.dma_start(out=s_itf[16 * k:16 * k + 16, :], in_=s_it16[:])
            nat_it = sp.tile([BC, 1], i32, tag="natit")
            nc.sync.dma_start(out=nat_it[:], in_=items[:, None])
            it_rm = rmask3(nat_it, "it")
            git = scr.tile([P, 4 * D], fp16, tag="git", bufs=1)
            nc.gpsimd.dma_gather(
                out_ap=git[:].rearrange("p (kk e) -> p kk e", kk=1),
                in_ap=ent_blk, idxs_ap=s_itf[:], num_idxs=BC, num_idxs_reg=BC,
                elem_size=256, single_packet=False)
            itrows = sp.tile([BC, D], fp16, tag="itrows")
            gitv = git[0:BC, :].rearrange("p (s r d) -> p s r d", s=1, r=4)
            nc.vector.tensor_copy(out=itrows[:].rearrange("p (s d) -> p s d", s=1),
                                  in_=gitv[:, :, 0, :])
            for kk in (1, 2, 3):
                nc.vector.copy_predicated(out=itrows[:].rearrange("p (s d) -> p s d", s=1),
                                          mask=bc(it_rm[kk - 1][:], D), data=gitv[:, :, kk, :])
            itrows32 = scr.tile([BC, D], fp32, tag="itrows32", bufs=1)
            nc.vector.tensor_copy(out=itrows32[:], in_=itrows[:])
            it_pt = pp.tile([P, P], fp32, space="PSUM", tag="y")
            it_p = it_pt[0:D, 0:BC]
            nc.tensor.transpose(out=it_p, in_=itrows32[:], identity=id128[0:BC, 0:BC])

            if DBG:
                nc.sync.dma_start(out=DBG["accu"][:, :], in_=acc128["u"][:])

            # ---------------- final: sigmoid(e_u . e_v) ----------------
            # fold acc128 halves (PE) then parity columns; subtract the 2T elu bias
            eu_p = pp1.tile([D, P], fp32, space="PSUM", tag="pp1f")
            nc.tensor.matmul(out=eu_p[:], lhsT=stack2[:], rhs=acc128["u"][:], start=True, stop=True)
            eu_sb = scr.tile([D, P], fp32, tag="fold_sb", bufs=1)
            nc.vector.tensor_copy(out=eu_sb[:], in_=eu_p[:])
            eu_s = cp.tile([D, BC], fp32, tag="eu_s")
            ev_half = eu_sb[:].rearrange("p (b two) -> p b two", two=2)
            nc.vector.tensor_tensor(out=eu_s[:], in0=ev_half[:, :, 0],
                                    in1=ev_half[:, :, 1], op=Alu.add)
            nc.vector.tensor_scalar(out=eu_s[:], in0=eu_s[:], scalar1=float(2 * T),
                                    scalar2=None, op0=Alu.subtract)
            ev_p = pp1.tile([D, P], fp32, space="PSUM", tag="pp1f")
            nc.tensor.matmul(out=ev_p[:], lhsT=stack2[:], rhs=acc128["v"][:], start=True, stop=True)
            ev_sb = scr.tile([D, P], fp32, tag="fold_sb", bufs=1)
            nc.vector.tensor_copy(out=ev_sb[:], in_=ev_p[:])
            ev_s = cp.tile([D, BC], fp32, tag="ev_s")
            ev_half2 = ev_sb[:].rearrange("p (b two) -> p b two", two=2)
            nc.vector.tensor_tensor(out=ev_s[:], in0=ev_half2[:, :, 0],
                                    in1=ev_half2[:, :, 1], op=Alu.add)
            nc.vector.tensor_scalar(out=ev_s[:], in0=ev_s[:], scalar1=float(2 * T),
                                    scalar2=None, op0=Alu.subtract)
            nc.vector.tensor_tensor(out=ev_s[:], in0=ev_s[:], in1=it_p, op=Alu.add)
            prod = scr.tile([D, BC], fp32, tag="prod", bufs=1)
            nc.vector.tensor_tensor(out=prod[:], in0=eu_s[:], in1=ev_s[:], op=Alu.mult)
            dot_p = pp1.tile([1, BC], fp32, space="PSUM", tag="pp1t")
            nc.tensor.matmul(out=dot_p[:], lhsT=ones64[:], rhs=prod[:], start=True, stop=True)
            sig = cp.tile([1, BC], fp32)
            nc.scalar.activation(sig[:], dot_p[:], Act.Sigmoid)
            nc.sync.dma_start(out=out_t[:, :], in_=sig[:])

    nc.compile()
    return nc


def _prep_inputs(inputs):
    """Build the 8 per-core input maps from full inputs."""
    f32 = np.float32
    ent = np.asarray(inputs["entity_emb"], f32)
    entH = np.ascontiguousarray(ent.astype(np.float16))
    rel = np.ascontiguousarray(np.asarray(inputs["relation_emb"], f32))
    Wg = np.ascontiguousarray(np.asarray(inputs["W_GAT"], f32))
    ag = np.ascontiguousarray(np.asarray(inputs["a_GAT"], f32))
    entH_pad = np.zeros((NE_PAD, D), np.float16)
    entH_pad[:NE] = entH

    def i32(x):
        return np.ascontiguousarray(np.asarray(x, np.int32))

    items = i32(inputs["items"])
    uh, ut = i32(inputs["user_h"]), i32(inputs["user_t"])
    unh, unr, unt = i32(inputs["user_nh"]), i32(inputs["user_nr"]), i32(inputs["user_nt"])
    ih, it_ = i32(inputs["item_h"]), i32(inputs["item_t"])
    inh, inr, int_ = i32(inputs["item_nh"]), i32(inputs["item_nr"]), i32(inputs["item_nt"])

    maps = []
    for c in range(NCORES):
        bs = slice(c * BC, (c + 1) * BC)
        m = {
            "entH": entH,
            "eslH": np.ascontiguousarray(entH_pad[c * EPC:(c + 1) * EPC]),
            "relation_emb": rel,
            "W_GAT": Wg,
            "a_GAT": ag,
            "items": items[bs],
            "h0_u": uh[0, bs].reshape(BT),
            "h0_i": ih[0, bs].reshape(BT),
        }
        for li in range(2):
            m[f"nh_u{li}"] = unh[li, bs].reshape(BT, NN)
            m[f"nr_u{li}"] = unr[li, bs].reshape(BT, NN)
            m[f"nt_u{li}"] = unt[li, bs].reshape(BT, NN)
            m[f"t_u{li}"] = ut[li, bs].reshape(BT)
            m[f"nh_i{li}"] = inh[li, bs].reshape(BT, NN)
            m[f"nr_i{li}"] = inr[li, bs].reshape(BT, NN)
            m[f"nt_i{li}"] = int_[li, bs].reshape(BT, NN)
            m[f"t_i{li}"] = it_[li, bs].reshape(BT)
        maps.append(m)
    return maps


def kernel(**inputs) -> np.ndarray:
    from concourse import bass_utils
    if "nc" not in _CACHE:
        _CACHE["nc"] = _build()
    nc = _CACHE["nc"]
    maps = _prep_inputs(inputs)
    res = bass_utils.run_bass_kernel_spmd(nc, maps, core_ids=list(range(NCORES)))
    return np.concatenate([res.results[c]["out"][0] for c in range(NCORES)]).astype(np.float32)
